# revision 1
# baseline (speedup 1.0000x reference)
"""CrossAttentionLayer Trainium2 kernel: 8-way batch-parallel, fp32r matmuls.

Per-core (batch element n) plan, activations kept transposed [C, L] in SBUF:
  phase A: load x [4096,512] -> PE-transpose to xT [512,4096]; mean over L via
           ones-matmul; one-hot outer product + AllReduce = gf all-gather.
  phase B: memory generator. mg1 computes only this core's 128-row slice of
           h (c_out shard); mg2 row-shard [128,65536] streams from HBM,
           partial products ReduceScatter'd -> this core's memory_params.
  phase C: k/v projections, elu features, per-head KV [d,v], Ksum block-diag.
  phase D: 8 chunks of 512 columns: q-proj -> elu -> z matmul -> 1/z ->
           PE-broadcast z -> per-head Q@KV -> scale -> merge -> LN1
           (partition-dim stats via ones-matmul, g/b folded into mlp_w1/bias)
           -> mlp1+relu -> mlp2 (natural [l,c] out) -> LN2 (free-dim stats via
           accum_out) -> +x residual -> store.

elu(x)+1 == relu(x) + exp(min(x,0)) exactly.
Z-normalizer: out_h = (Q_h @ KV_h) / (Q_h . Ksum_h + 1e-6); the /64 on V and
*64 at the end cancel.
"""

import numpy as np

import concourse.bacc as bacc
import concourse.mybir as mybir
import concourse.tile as tile
from concourse.bass_utils import run_bass_kernel_spmd

F32 = mybir.dt.float32
F32R = mybir.dt.float32r
ALU = mybir.AluOpType
ACTF = mybir.ActivationFunctionType

N_CORES = 8
L = 4096
C = 512
C2 = 1024
NHEAD = 8
HD = 64
KV = 64
CH = 512          # chunk of L columns per main-loop iteration
NCH = L // CH     # 8
CT = C // 128     # 4 c-tiles
EPS_Z = 1e-6
EPS_LN = 1e-5

_CACHE = {}
NO_CC = False      # debug: replace collectives with local DMA
REPEAT = 1         # run the whole pipeline K times (for differential timing)
MAIN_LEVEL = 7     # debug: how much of phase D to emit
SKIP_MAIN = False  # debug: skip phase D


def r(ap):
    return ap.bitcast(F32R)



def build_nc(l_rows=None):
    global L, NCH
    if l_rows is not None:
        L = l_rows
        NCH = L // CH
    nc = bacc.Bacc("TRN2", target_bir_lowering=False, debug=False,
                   num_devices=N_CORES)
    P = {}
    for name, shape in [
        ("x", [L, C]),
        ("qw", [C, C]), ("kw", [C, C]), ("vw", [C, C]), ("mw", [C, C]),
        ("w1", [C2, C2]), ("w2", [C2, C]),
        ("mgw1s", [C + 1, 128]),          # per-core c_out shard of [mg_w1; mg_b1]
        ("mgw2s", [128, C * KV * 2]),     # per-core row shard of mg_w2
        ("b2m", [128, CH]),               # mg_b2 reshaped [128, 512]
        ("onehot", [1, 128]),   # one-hot row zero-padded to 128
        ("bmask", [8, C]),                # head indicator, lhsT for z broadcast
        ("ident", [128, 128]),
        ("ones128", [128, 128]),  # col 0 ones, rest zero
        ("ones_row", [1, 128]),
        ("n1g", [C]), ("n1b", [C]), ("n2g", [C]), ("n2b", [C]),
    ]:
        P[name] = nc.declare_dram_parameter(name, shape, F32, isOutput=False)
    out_ext = nc.declare_dram_parameter("out", [L, C], F32, isOutput=True)

    groups = [list(range(N_CORES))]

    with tile.TileContext(nc) as tc:
        with (
            tc.tile_pool(name="res", bufs=1) as res,          # persistent tiles
            tc.tile_pool(name="wrk", bufs=1) as wrk,          # main-loop tiles
            tc.tile_pool(name="ps", bufs=6, space="PSUM") as ps,
            tc.tile_pool(name="psx", bufs=2, space="PSUM") as psx,
            tc.tile_pool(name="dram", bufs=1, space="DRAM") as dram,
        ):
            def pst(name, shape=(128, CH)):
                return ps.tile(list(shape), F32, name=name, tag="ps")

            def psa(name, shape=(1, CH)):
                return psx.tile(list(shape), F32, name=name, tag="psx")

            def scr(name):  # rotating [128, CH] scratch
                return wrk.tile([128, CH], F32, name=name, tag="scr", bufs=5)

            def row(name, p=1):  # rotating [1|8, CH] scratch
                return wrk.tile([p, CH], F32, name=name, tag="row", bufs=4)

            def tiny(name):  # [128, 1] scratch
                return wrk.tile([128, 1], F32, name=name, tag="tiny", bufs=8)

            # ---- constants ----
            ident = res.tile([128, 128], F32)
            nc.sync.dma_start(r(ident[:]), r(P["ident"][:]))
            ones128 = res.tile([128, 128], F32)
            nc.sync.dma_start(r(ones128[:]), r(P["ones128"][:]))
            ones_row = res.tile([1, 128], F32)
            nc.sync.dma_start(r(ones_row[:]), r(P["ones_row"][:]))
            bmask_sb = res.tile([8, C], F32)
            nc.sync.dma_start(r(bmask_sb[:]), r(P["bmask"][:]))
            eps1 = res.tile([1, 1], F32)
            nc.vector.memset(eps1[:], EPS_LN)
            eps2 = res.tile([128, 1], F32)
            nc.vector.memset(eps2[:], EPS_LN)

            # ---- resident weights ----
            qw_sb = [res.tile([128, C], F32, name=f"qw{k}") for k in range(CT)]
            mw_sb = [res.tile([128, C], F32, name=f"mw{k}") for k in range(CT)]
            for k in range(CT):
                nc.sync.dma_start(r(qw_sb[k][:]), r(P["qw"][k * 128:(k + 1) * 128, :]))
                nc.sync.dma_start(r(mw_sb[k][:]), r(P["mw"][k * 128:(k + 1) * 128, :]))
            w1_sb = [res.tile([128, C2], F32, name=f"w1_{k}") for k in range(8)]
            for k in range(8):
                nc.sync.dma_start(r(w1_sb[k][:]), r(P["w1"][k * 128:(k + 1) * 128, :]))
            w2_sb = [res.tile([128, C], F32, name=f"w2_{k}") for k in range(8)]
            for k in range(8):
                nc.sync.dma_start(r(w2_sb[k][:]), r(P["w2"][k * 128:(k + 1) * 128, :]))

            gB = res.tile([128, C], F32)
            bB = res.tile([128, C], F32)
            bv_sb = res.tile([128, 8], F32)

            with tc.tile_pool(name="setup", bufs=1) as setup:
                # norm params
                n1g_ct = [setup.tile([128, 1], F32, name=f"n1g{k}") for k in range(CT)]
                n1b_ct = [setup.tile([128, 1], F32, name=f"n1b{k}") for k in range(CT)]
                for k in range(CT):
                    nc.sync.dma_start(r(n1g_ct[k][:]),
                                      r(P["n1g"][k * 128:(k + 1) * 128, None]))
                    nc.sync.dma_start(r(n1b_ct[k][:]),
                                      r(P["n1b"][k * 128:(k + 1) * 128, None]))
                n2g_row = setup.tile([1, C], F32)
                nc.sync.dma_start(r(n2g_row[:]), r(P["n2g"][None, :]))
                n2b_row = setup.tile([1, C], F32)
                nc.sync.dma_start(r(n2b_row[:]), r(P["n2b"][None, :]))

                # gB/bB: n2 gamma/beta broadcast to 128 partitions (K=1 matmul)
                pgb = pst("pgb", (128, C))
                nc.tensor.matmul(pgb[:], r(ones_row[:]), r(n2g_row[:]),
                                 start=True, stop=True)
                nc.scalar.copy(gB[:], pgb[:])
                pbb = pst("pbb", (128, C))
                nc.tensor.matmul(pbb[:], r(ones_row[:]), r(n2b_row[:]),
                                 start=True, stop=True)
                nc.scalar.copy(bB[:], pbb[:])

                # bvec = n1b @ w1[512:1024]  (bias fold of LN1 beta into mlp1)
                n1b128 = [setup.tile([128, 128], F32, name=f"n1b128_{k}")
                          for k in range(CT)]
                for k in range(CT):
                    nc.vector.memset(n1b128[k][:], 0.0)
                    nc.vector.tensor_copy(r(n1b128[k][:, 0:1]), n1b_ct[k][:])
                bv_row = setup.tile([1, C2], F32)
                for half in range(2):
                    pbv = pst("pbv", (128, C))
                    for k in range(CT):
                        nc.tensor.matmul(pbv[:], r(n1b128[k][:]),
                                         r(w1_sb[4 + k][:, half * C:(half + 1) * C]),
                                         start=(k == 0), stop=(k == CT - 1))
                    nc.scalar.copy(bv_row[:, half * C:(half + 1) * C], pbv[0:1, :])
                for m in range(8):
                    pbt = pst("pbt", (128, 128))
                    nc.tensor.transpose(pbt[:, 0:1],
                                        bv_row[:, m * 128:(m + 1) * 128],
                                        ident[0:1, 0:1])
                    nc.scalar.copy(bv_sb[:, m:m + 1], pbt[:, 0:1])

                # fold LN1 gamma into lower half of w1 (rows 512:1024)
                for k in range(CT):
                    nc.vector.tensor_scalar(r(w1_sb[4 + k][:]), w1_sb[4 + k][:],
                                            n1g_ct[k][:], None, op0=ALU.mult)

            # ---- phase A: stream x, accumulate mean; AllReduce one-hot gather ----
            GF_T = [res.tile([128, 8], F32, name=f"GFT{t}") for t in range(CT)]
            H_T = res.tile([128, 128], F32)
            for _rep in range(REPEAT):
              with tc.tile_pool(name="phA", bufs=1) as phA:
                  pgf = pst("pgf", (128, C))
                  for lt in range(L // 128):
                      xn = phA.tile([128, C], F32, name="xn", tag="xn", bufs=3)
                      nc.sync.dma_start(r(xn[:]), r(P["x"][lt * 128:(lt + 1) * 128, :]))
                      nc.tensor.matmul(pgf[:], r(ones128[:]), r(xn[:]),
                                       start=(lt == 0), stop=(lt == L // 128 - 1))
                  gf_row = phA.tile([1, C], F32)
                  nc.scalar.activation(r(gf_row[:]), pgf[0:1, :], ACTF.Copy,
                                       scale=1.0 / L)
                  oh_sb = phA.tile([1, 128], F32)
                  nc.sync.dma_start(r(oh_sb[:]), r(P["onehot"][:]))

                  pout = pst("pout", (128, C))
                  nc.tensor.matmul(pout[:], r(oh_sb[:]), r(gf_row[:]),
                                   start=True, stop=True)
                  gf_loc = phA.tile([8, C], F32)
                  nc.scalar.copy(gf_loc[:], pout[0:8, :])
                  ar_in = dram.tile([8, C], F32)
                  ar_out = dram.tile([8, C], F32, addr_space="Shared")
                  nc.sync.dma_start(ar_in[:], gf_loc[:])
                  if NO_CC:
                      nc.sync.dma_start(ar_out[:], ar_in[:])
                  else:
                      nc.gpsimd.collective_compute(
                          "AllReduce", ALU.add, replica_groups=groups,
                          ins=[ar_in.opt()], outs=[ar_out.opt()])
                  GF_cat = phA.tile([8, C], F32)
                  nc.sync.dma_start(r(GF_cat[:]), r(ar_out[:]))
                  for t in range(CT):
                      pt = pst("ptg", (128, 128))
                      nc.tensor.transpose(r(pt[:, 0:8]),
                                          r(GF_cat[:, t * 128:(t + 1) * 128]),
                                          r(ident[0:8, 0:8]))
                      nc.scalar.copy(r(GF_T[t][:]), pt[:, 0:8])

                  # mg1 shard: H_T = relu(mg_w1_shard.T @ GF + b1_shard)
                  mg1w = [phA.tile([128, 128], F32, name=f"mg1w{k}") for k in range(CT)]
                  for k in range(CT):
                      nc.sync.dma_start(r(mg1w[k][:]),
                                        r(P["mgw1s"][k * 128:(k + 1) * 128, :]))
                  mg1b = phA.tile([1, 128], F32)
                  nc.sync.dma_start(r(mg1b[:]), r(P["mgw1s"][C:C + 1, :]))
                  ones8 = phA.tile([1, 8], F32)
                  nc.vector.memset(ones8[:], 1.0)

                  ph = pst("ph", (128, 8))
                  for k in range(CT):
                      nc.tensor.matmul(ph[:], r(mg1w[k][:]), r(GF_T[k][:]),
                                       start=(k == 0), stop=False)
                  nc.tensor.matmul(ph[:], r(mg1b[:]), r(ones8[:]),
                                   start=False, stop=True)
                  nc.vector.memset(H_T[:], 0.0)
                  nc.scalar.activation(r(H_T[:, 0:8]), ph[:], ACTF.Relu)

              # ---- phase B: mg2 row-shard stream + ReduceScatter ----
              mp_sb = res.tile([128, C], F32)
              with tc.tile_pool(name="phB", bufs=1) as phB:
                  rs_in = dram.tile([8, C * KV * 2], F32)
                  rs_out = dram.tile([C * KV * 2], F32)
                  for chn in range(128):
                      pm = pst("pmg2")
                      wt = phB.tile([128, CH], F32, name="w2s", tag="w2s", bufs=4)
                      nc.sync.dma_start(r(wt[:]),
                                        r(P["mgw2s"][:, chn * CH:(chn + 1) * CH]))
                      nc.tensor.matmul(pm[:], r(H_T[:]), r(wt[:]),
                                       start=True, stop=True)
                      mg_sb = phB.tile([8, CH], F32, name="mg_sb", tag="mg", bufs=2)
                      if chn % 2 == 0:
                          nc.scalar.copy(mg_sb[:], pm[0:8, :])
                      else:
                          nc.vector.tensor_copy(mg_sb[:], pm[0:8, :])
                      nc.sync.dma_start(rs_in[:, chn * CH:(chn + 1) * CH], mg_sb[:])
                  if NO_CC:
                      nc.sync.dma_start(rs_out[:], rs_in[0, :])
                  else:
                      nc.gpsimd.collective_compute(
                          "ReduceScatter", ALU.add, replica_groups=groups,
                          ins=[rs_in.opt()], outs=[rs_out[:]])
                  mp_raw = phB.tile([128, C], F32)
                  nc.sync.dma_start(mp_raw[:], rs_out[:].rearrange("(p c) -> p c", c=C))
                  b2m = phB.tile([128, C], F32)
                  nc.sync.dma_start(b2m[:], P["b2m"][:])
                  nc.vector.tensor_tensor(r(mp_sb[:]), mp_raw[:], b2m[:], op=ALU.add)

              # ---- phase C: k/v projections, elu K, KV, Ksum block-diag ----
              K_feat = res.tile([64, C], F32)
              V_sb = res.tile([64, C], F32)
              KV_pe = res.tile([64, C], F32)
              KV_po = res.tile([64, C], F32)
              BD_col = [res.tile([128, 128], F32, name=f"BDc{t}") for t in range(CT)]
              with tc.tile_pool(name="phC", bufs=1) as phC:
                  mp_T = [phC.tile([128, 128], F32, name=f"mpT{t}") for t in range(CT)]
                  for t in range(CT):
                      pt = pst("ptm", (128, 128))
                      nc.tensor.transpose(r(pt[:]), r(mp_sb[:, t * 128:(t + 1) * 128]),
                                          r(ident[:]))
                      nc.scalar.copy(r(mp_T[t][:]), pt[:])

                  pk = pst("pk", (64, C))
                  pv = pst("pv", (64, C))
                  for k in range(CT):
                      kwt = phC.tile([128, C], F32, name="kwt", tag="kvw", bufs=3)
                      nc.sync.dma_start(r(kwt[:]), r(P["kw"][k * 128:(k + 1) * 128, :]))
                      vwt = phC.tile([128, C], F32, name="vwt", tag="kvw", bufs=3)
                      nc.sync.dma_start(r(vwt[:]), r(P["vw"][k * 128:(k + 1) * 128, :]))
                      nc.tensor.matmul(pk[:], r(mp_T[k][:, 0:KV]), r(kwt[:]),
                                       start=(k == 0), stop=(k == CT - 1))
                      nc.tensor.matmul(pv[:], r(mp_T[k][:, KV:2 * KV]), r(vwt[:]),
                                       start=(k == 0), stop=(k == CT - 1))
                  # elu(K)+1 = relu(K) + exp(min(K,0))
                  km = phC.tile([64, C], F32)
                  nc.vector.tensor_scalar(km[:], pk[:], 0.0, None, op0=ALU.min)
                  ke = phC.tile([64, C], F32)
                  nc.scalar.activation(ke[:], km[:], ACTF.Exp)
                  nc.vector.scalar_tensor_tensor(r(K_feat[:]), pk[:], 0.0, ke[:],
                                                 op0=ALU.max, op1=ALU.add)
                  nc.scalar.copy(r(V_sb[:]), pv[:])

                  pkv = pst("pkv", (64, C))
                  for h in range(NHEAD):
                      nc.tensor.matmul(pkv[:, h * HD:(h + 1) * HD],
                                       r(K_feat[:, h * HD:(h + 1) * HD]),
                                       r(V_sb[:, h * HD:(h + 1) * HD]),
                                       start=True, stop=True)
                  # padded KV: even heads at cols [t*128, t*128+64) of KV_pe (base 0),
                  # odd heads at cols [t*128+64, (t+1)*128) of KV_po rows 64:128.
                  nc.vector.memset(KV_pe[:], 0.0)
                  nc.vector.memset(KV_po[:], 0.0)
                  for t in range(CT):
                      h0, h1 = 2 * t, 2 * t + 1
                      nc.scalar.copy(r(KV_pe[:, t * 128:t * 128 + 64]),
                                     pkv[:, h0 * HD:(h0 + 1) * HD])
                      nc.scalar.copy(r(KV_po[:, t * 128 + 64:(t + 1) * 128]),
                                     pkv[:, h1 * HD:(h1 + 1) * HD])

                  pks = pst("pks", (128, C))
                  nc.tensor.matmul(pks[:], r(ones128[0:64, :]), r(K_feat[:]),
                                   start=True, stop=True)
                  ks_row = phC.tile([1, C], F32)
                  nc.scalar.copy(r(ks_row[:]), pks[0:1, :])
                  pksb = pst("pksb", (128, C))
                  nc.tensor.matmul(pksb[:], r(ones_row[:]), r(ks_row[:]),
                                   start=True, stop=True)
                  BD = phC.tile([8, C], F32)
                  nc.vector.tensor_tensor(r(BD[:]), pksb[0:8, :], bmask_sb[:],
                                          op=ALU.mult)
                  for t in range(CT):
                      pt = pst("ptb", (128, 128))
                      nc.tensor.transpose(r(pt[:, 0:8]), r(BD[:, t * 128:(t + 1) * 128]),
                                          r(ident[0:8, 0:8]))
                      nc.vector.memset(BD_col[t][:], 0.0)
                      nc.scalar.copy(r(BD_col[t][:, 0:8]), pt[:, 0:8])

              # ---- phase D: main chunk loop ----
              for ch in range(NCH if not SKIP_MAIN else 0):
                  # load x chunk (natural), transpose to xTc
                  xn_c = [wrk.tile([128, C], F32, name=f"xn{lt}", bufs=2)
                          for lt in range(4)]
                  xTc = [wrk.tile([128, CH], F32, name=f"xT{k}", bufs=2)
                         for k in range(CT)]
                  for lt in range(4):
                      row0 = ch * CH + lt * 128
                      nc.sync.dma_start(r(xn_c[lt][:]), r(P["x"][row0:row0 + 128, :]))
                      for t in range(CT):
                          pt = pst("ptr", (128, 128))
                          nc.tensor.transpose(r(pt[:]),
                                              r(xn_c[lt][:, t * 128:(t + 1) * 128]),
                                              r(ident[:]))
                          nc.scalar.copy(r(xTc[t][:, lt * 128:(lt + 1) * 128]), pt[:])

                  if MAIN_LEVEL < 2:
                      for lt in range(4):
                          row0 = ch * CH + lt * 128
                          nc.sync.dma_start(out_ext[row0:row0 + 128, :], xn_c[lt][:])
                      continue
                  # q-projection + elu
                  Qf = [wrk.tile([128, CH], F32, name=f"qf{t}", bufs=1)
                        for t in range(CT)]
                  Qfh = [wrk.tile([64, CH], F32, name=f"qfh{t}", bufs=1)
                         for t in range(CT)]
                  for t in range(CT):
                      pq = pst("pq")
                      for k in range(CT):
                          nc.tensor.matmul(pq[:], r(qw_sb[k][:, t * 128:(t + 1) * 128]),
                                           r(xTc[k][:]),
                                           start=(k == 0), stop=(k == CT - 1))
                      qm = scr("qm")
                      nc.vector.tensor_scalar(qm[:], pq[:], 0.0, None, op0=ALU.min)
                      qe = scr("qe")
                      nc.scalar.activation(qe[:], qm[:], ACTF.Exp)
                      nc.vector.scalar_tensor_tensor(r(Qf[t][:]), pq[:], 0.0, qe[:],
                                                     op0=ALU.max, op1=ALU.add)
                      nc.vector.tensor_copy(r(Qfh[t][:]), Qf[t][64:128, :])
                  if MAIN_LEVEL < 3:
                      for lt in range(4):
                          row0 = ch * CH + lt * 128
                          nc.sync.dma_start(out_ext[row0:row0 + 128, :],
                                            Qf[lt][:, 0:C])
                      continue
                  # z = BD_col.T @ Qf ; zr = 1/(z+eps)
                  pz = pst("pz")
                  for t in range(CT):
                      nc.tensor.matmul(pz[:], r(BD_col[t][:]), r(Qf[t][:]),
                                       start=(t == 0), stop=(t == CT - 1))
                  zpe = row("zpe", 8)
                  nc.vector.tensor_scalar(zpe[:], pz[0:8, :], EPS_Z, None, op0=ALU.add)
                  zr = row("zr", 8)
                  with nc.allow_low_precision(reason="f32r label for matmul rhs"):
                      nc.vector.reciprocal(r(zr[:]), zpe[:])
                  if MAIN_LEVEL < 4:
                      for lt in range(4):
                          row0 = ch * CH + lt * 128
                          nc.sync.dma_start(out_ext[row0:row0 + 128, :],
                                            zr[0:8, 0:C].rearrange("a b -> a b") if False else Qf[lt][:, 0:C])
                      continue
                  # broadcast z to head rows; attention; scale
                  msgT = [wrk.tile([128, CH], F32, name=f"msgT{t}", bufs=1)
                          for t in range(CT)]
                  for t in range(CT):
                      pzb = pst("pzb")
                      nc.tensor.matmul(pzb[:], r(bmask_sb[:, t * 128:(t + 1) * 128]),
                                       r(zr[:]), start=True, stop=True)
                      zbs = scr("zbs")
                      nc.scalar.copy(zbs[:], pzb[:])
                      pat = pst("pat")
                      nc.tensor.matmul(pat[:],
                                       r(KV_pe[:, t * 128:(t + 1) * 128]),
                                       r(Qf[t][0:64, :]), start=True, stop=False)
                      nc.tensor.matmul(pat[:],
                                       r(KV_po[:, t * 128:(t + 1) * 128]),
                                       r(Qfh[t][:]), start=False, stop=True)
                      nc.vector.scalar_tensor_tensor(r(msgT[t][:]), pat[:], 0.0,
                                                     zbs[:], op0=ALU.add,
                                                     op1=ALU.mult)
                  # merge matmul + LN1 stats
                  ms = [wrk.tile([128, CH], F32, name=f"ms{t}", bufs=1)
                        for t in range(CT)]
                  ps1 = pst("ps1")
                  ps2 = pst("ps2")
                  for t in range(CT):
                      pmg = pst("pmrg")
                      for k in range(CT):
                          nc.tensor.matmul(pmg[:], r(mw_sb[k][:, t * 128:(t + 1) * 128]),
                                           r(msgT[k][:]),
                                           start=(k == 0), stop=(k == CT - 1))
                      nc.scalar.copy(r(ms[t][:]), pmg[:])
                      sq = scr("sq")
                      nc.scalar.activation(r(sq[:]), ms[t][:], ACTF.Square)
                      nc.tensor.matmul(ps1[:], r(ones128[:]), r(ms[t][:]),
                                       start=(t == 0), stop=(t == CT - 1))
                      nc.tensor.matmul(ps2[:], r(ones128[:]), r(sq[:]),
                                       start=(t == 0), stop=(t == CT - 1))
                  # LN1 stats
                  mu1 = row("mu1")
                  nc.scalar.activation(r(mu1[:]), ps1[0:1, :], ACTF.Copy, scale=1.0 / C)
                  mu1s = row("mu1s")
                  nc.scalar.activation(mu1s[:], mu1[:], ACTF.Square)
                  var1 = row("var1")
                  nc.vector.scalar_tensor_tensor(var1[:], ps2[0:1, :], 1.0 / C, mu1s[:],
                                                 op0=ALU.mult, op1=ALU.subtract)
                  sd1 = row("sd1")
                  nc.scalar.activation(sd1[:], var1[:], ACTF.Sqrt, bias=eps1[:])
                  A1 = row("A1")
                  with nc.allow_low_precision(reason="f32r label for matmul rhs"):
                      nc.vector.reciprocal(r(A1[:]), sd1[:])
                  pA = pst("pA")
                  nc.tensor.matmul(pA[:], r(ones_row[:]), r(A1[:]), start=True, stop=True)
                  pB = pst("pB")
                  nc.tensor.matmul(pB[:], r(ones_row[:]), r(mu1[:].bitcast(F32R)),
                                   start=True, stop=True)
                  muB = scr("muB")
                  nc.scalar.copy(muB[:], pB[:])
                  ln1 = [wrk.tile([128, CH], F32, name=f"ln1_{t}", bufs=1)
                         for t in range(CT)]
                  for t in range(CT):
                      df = scr("df")
                      nc.vector.tensor_tensor(df[:], ms[t][:], muB[:], op=ALU.subtract)
                      nc.vector.tensor_tensor(r(ln1[t][:]), df[:], pA[:], op=ALU.mult)
                  if MAIN_LEVEL < 6:
                      for lt in range(4):
                          row0 = ch * CH + lt * 128
                          nc.sync.dma_start(out_ext[row0:row0 + 128, :],
                                            ln1[lt][:, 0:C])
                      continue
                  # mlp1 (two groups of 4 c_out tiles) + relu + bias
                  hid = [wrk.tile([128, CH], F32, name=f"hid{m}", bufs=1)
                         for m in range(8)]
                  for grp in range(2):
                      for mi in range(4):
                          m = grp * 4 + mi
                          ph1 = pst(f"ph1_{mi}")
                          for k in range(CT):
                              nc.tensor.matmul(ph1[:],
                                               r(w1_sb[k][:, m * 128:(m + 1) * 128]),
                                               r(xTc[k][:]),
                                               start=(k == 0), stop=False)
                          for k in range(CT):
                              nc.tensor.matmul(ph1[:],
                                               r(w1_sb[4 + k][:, m * 128:(m + 1) * 128]),
                                               r(ln1[k][:]),
                                               start=False, stop=(k == CT - 1))
                          nc.scalar.activation(r(hid[m][:]), ph1[:], ACTF.Relu,
                                               bias=bv_sb[:, m:m + 1])
                  if MAIN_LEVEL < 7:
                      for lt in range(4):
                          row0 = ch * CH + lt * 128
                          nc.sync.dma_start(out_ext[row0:row0 + 128, :],
                                            hid[lt][:, 0:C])
                      continue
                  # mlp2 natural out + LN2 + residual, per 128-row l-tile
                  for lt in range(CH // 128):
                      po = pst("po")
                      for m in range(8):
                          nc.tensor.matmul(po[:], r(hid[m][:, lt * 128:(lt + 1) * 128]),
                                           r(w2_sb[m][:]),
                                           start=(m == 0), stop=(m == 7))
                      o_sb = scr("o_sb")
                      s1 = tiny("s1")
                      nc.scalar.activation(o_sb[:], po[:], ACTF.Copy, accum_out=s1[:])
                      sq2 = scr("sq2")
                      s2 = tiny("s2")
                      nc.scalar.activation(sq2[:], o_sb[:], ACTF.Square,
                                           accum_out=s2[:])
                      mu = tiny("mu")
                      nc.vector.tensor_scalar(mu[:], s1[:], 1.0 / C, None, op0=ALU.mult)
                      mu2 = tiny("mu2")
                      nc.vector.tensor_tensor(mu2[:], mu[:], mu[:], op=ALU.mult)
                      var = tiny("var")
                      nc.vector.scalar_tensor_tensor(var[:], s2[:], 1.0 / C, mu2[:],
                                                     op0=ALU.mult, op1=ALU.subtract)
                      sdv = tiny("sdv")
                      nc.scalar.activation(sdv[:], var[:], ACTF.Sqrt, bias=eps2[:])
                      Av = tiny("Av")
                      nc.vector.reciprocal(Av[:], sdv[:])
                      u = scr("u")
                      nc.vector.scalar_tensor_tensor(u[:], o_sb[:], mu[:], gB[:],
                                                     op0=ALU.subtract, op1=ALU.mult)
                      y1 = scr("y1")
                      nc.vector.scalar_tensor_tensor(y1[:], u[:], Av[:], xn_c[lt][:],
                                                     op0=ALU.mult, op1=ALU.add)
                      y = scr("y")
                      nc.vector.tensor_tensor(y[:], y1[:], bB[:], op=ALU.add)
                      row0 = ch * CH + lt * 128
                      nc.sync.dma_start(out_ext[row0:row0 + 128, :], y[:])

              if SKIP_MAIN:
                  for lt in range(L // 128):
                      zt = wrk.tile([128, C], F32, name="zt", tag="scr", bufs=2)
                      nc.vector.tensor_scalar(zt[:], mp_sb[0:128, :], 1.0, None,
                                              op0=ALU.mult)
                      nc.sync.dma_start(out_ext[lt * 128:(lt + 1) * 128, :], zt[:])

    nc.compile()
    return nc



def _prep_in_maps(inputs):
    x = np.ascontiguousarray(inputs["x"], dtype=np.float32)
    mg_w1 = np.asarray(inputs["mg_w1"], dtype=np.float32)
    mg_b1 = np.asarray(inputs["mg_b1"], dtype=np.float32)
    mg_w2 = np.asarray(inputs["mg_w2"], dtype=np.float32)
    mg_b2 = np.asarray(inputs["mg_b2"], dtype=np.float32)

    mgw1_aug = np.concatenate([mg_w1, mg_b1[None, :]], axis=0)  # [513, 1024]

    b2m = np.ascontiguousarray(mg_b2.reshape(128, CH).astype(np.float32))

    bmask = np.zeros((8, C), dtype=np.float32)
    for h in range(NHEAD):
        bmask[h, h * HD:(h + 1) * HD] = 1.0
    ones128 = np.zeros((128, 128), dtype=np.float32)
    ones128[:, 0] = 1.0

    common = {
        "qw": np.ascontiguousarray(inputs["q_w"], dtype=np.float32),
        "kw": np.ascontiguousarray(inputs["k_w"], dtype=np.float32),
        "vw": np.ascontiguousarray(inputs["v_w"], dtype=np.float32),
        "mw": np.ascontiguousarray(inputs["merge_w"], dtype=np.float32),
        "w1": np.ascontiguousarray(inputs["mlp_w1"], dtype=np.float32),
        "w2": np.ascontiguousarray(inputs["mlp_w2"], dtype=np.float32),
        "b2m": b2m,
        "bmask": bmask,
        "ident": np.eye(128, dtype=np.float32),
        "ones128": ones128,
        "ones_row": np.ones((1, 128), dtype=np.float32),
        "n1g": np.ascontiguousarray(inputs["norm1_g"], dtype=np.float32),
        "n1b": np.ascontiguousarray(inputs["norm1_b"], dtype=np.float32),
        "n2g": np.ascontiguousarray(inputs["norm2_g"], dtype=np.float32),
        "n2b": np.ascontiguousarray(inputs["norm2_b"], dtype=np.float32),
    }
    in_maps = []
    for n in range(N_CORES):
        m = dict(common)
        m["x"] = np.ascontiguousarray(x[n])
        m["mgw1s"] = np.ascontiguousarray(mgw1_aug[:, n * 128:(n + 1) * 128])
        m["mgw2s"] = np.ascontiguousarray(mg_w2[n * 128:(n + 1) * 128, :])
        oh = np.zeros((1, 128), dtype=np.float32)
        oh[0, n] = 1.0
        m["onehot"] = oh
        in_maps.append(m)
    return in_maps


def kernel(**inputs):
    if "nc" not in _CACHE:
        _CACHE["nc"] = build_nc()
    nc = _CACHE["nc"]
    in_maps = _prep_in_maps(inputs)
    res = run_bass_kernel_spmd(nc, in_maps, list(range(N_CORES)))
    out = np.stack([res.results[n]["out"] for n in range(N_CORES)], axis=0)
    return out.astype(np.float32)

